# revision 17
# baseline (speedup 1.0000x reference)
"""Dilated attention Trainium2 kernel (combined-weights formulation).

Problem: for each (batch, segment) pair, and each dilation rate r in {1,2,4,8}:
  out_seg[::r] += softmax(Q_seg[::r] @ K_seg[::r].T) @ V_seg[::r]

Key algebra: with q/k/v rows PERMUTED so each rate's strided index set becomes
a contiguous prefix (residue classes mod 8 ordered as {0},{4},{2,6},{odd}),
every rate-r score matrix is the leading L_r x L_r block of the ONE full
2048x2048 score matrix S.  Softmax is shift-invariant per row, so a single
exp table E = exp(S - blockmax) serves all rates; each rate only needs its
own row-normalizer.  Summing the per-rate softmax(P_r)@V contributions gives
  out[p] = (E[p, :] * c_{b(j)}[p]) @ V        (one full-width PV matmul)
where c_b[p] collapses the active rates' 1/Z_r weights per column block.
PE work drops from 2.82 to 2.12 full-matmul units and the per-rate DRAM
scratch round-trip disappears.

Numerics: exp biases are per-column-block maxes with block edges on the rate
boundaries (256/512/1024), so every rate's dominant weights are O(1) in fp16.
The combine coefficients c (<= 4.0) are built in f32 from block sums and
folded into the PE transpose by replacing the identity with diag(c).

Sharding: B=2 x n_seg=4 = 8 independent (batch, segment) pairs -> one per core.
"""

import sys

if "/opt/trn_rl_repo" not in sys.path:
    sys.path.insert(0, "/opt/trn_rl_repo")

import numpy as np

import concourse.bass as bass
import concourse.mybir as mybir
from concourse import tile
from concourse.masks import make_identity
from concourse.bass_utils import run_bass_kernel_spmd

SEG_LEN = 2048
D = 1024
P = 128
NDCH = D // P  # 8 d-chunks of 128
F16 = mybir.dt.float16
F32 = mybir.dt.float32

# bias blocks: aligned to the rate boundaries 256/512/1024
BIAS_BLOCKS = [(0, 256), (256, 512), (512, 1024), (1024, 1536), (1536, 2048)]
# j-tile (128 wide) -> bias block index
GROUP_OF_JT = [0, 0, 1, 1, 2, 2, 2, 2, 3, 3, 3, 3, 4, 4, 4, 4]
# rate -> bias blocks it spans (rate 8 handled separately: single block 0 half)
RATE_BLOCKS = {1: 5, 2: 3, 4: 2}  # number of leading bias blocks

_ws_ctr = [0]


def _split_multi_waits(nc):
    """walrus in this env accepts only ONE sync-wait per instruction; move
    extras onto same-engine NoOps inserted right before the instruction."""
    for f in nc.m.functions:
        for b in f.blocks:
            out, changed = [], False
            for inst in b.instructions:
                si = inst.sync_info
                if si is not None and si.on_wait and len(si.on_wait) > 1:
                    waits = list(si.on_wait)
                    for w in waits[:-1]:
                        nop = mybir.InstNoOp(
                            name=f"waitsplit_{_ws_ctr[0]}", ins=[], outs=[]
                        )
                        _ws_ctr[0] += 1
                        nop.engine = inst.engine
                        nop.sync_info = mybir.SyncInfo(on_wait=[w], on_update=[])
                        out.append(nop)
                    si.on_wait = [waits[-1]]
                    changed = True
                out.append(inst)
            if changed:
                b.instructions = out


def build_kernel():
    # note: --enable-ldw-opt=true crashes the device (NRT_EXEC_UNIT_UNRECOVERABLE)
    # note: nc.scalar-issued xbar-transpose DMAs return wrong data in this env
    nc = bass.Bass()
    # host uploads Q,K pre-transposed ([d, l]), pre-permuted and pre-cast
    QTd = nc.dram_tensor("QT", (D, SEG_LEN), F16, kind="ExternalInput")
    KTd = nc.dram_tensor("KT", (D, SEG_LEN), F16, kind="ExternalInput")
    V = nc.dram_tensor("V", (SEG_LEN, D), F16, kind="ExternalInput")
    O = nc.dram_tensor("O", (SEG_LEN, D), F32, kind="ExternalOutput")

    with tile.TileContext(nc) as tc:
        with (
            tc.tile_pool(name="qkt", bufs=1) as qkt_pool,
            tc.tile_pool(name="vp", bufs=1) as v_pool,
            tc.tile_pool(name="ep", bufs=3) as e_pool,
            tc.tile_pool(name="wt", bufs=3) as wt_pool,
            tc.tile_pool(name="dg", bufs=3) as dg_pool,
            tc.tile_pool(name="op", bufs=3) as o_pool,
            tc.tile_pool(name="st", bufs=8) as stat_pool,
            tc.tile_pool(name="misc", bufs=1) as misc_pool,
            tc.tile_pool(name="spsum", bufs=4, space="PSUM") as s_psum,
            tc.tile_pool(name="opsum", bufs=3, space="PSUM") as o_psum,
        ):
            # ---- loads: KT first (tile 0 needs all keys), then QT first half,
            # then V (needed at first PV), then QT second half
            KT = [
                qkt_pool.tile([P, SEG_LEN], F16, tag=f"KT{c}", name=f"KT{c}")
                for c in range(NDCH)
            ]
            QTh = [
                [
                    qkt_pool.tile([P, SEG_LEN // 2], F16, tag=f"QT{h}_{c}",
                                  name=f"QT{h}_{c}")
                    for c in range(NDCH)
                ]
                for h in range(2)
            ]
            dma_engines = [nc.sync, nc.scalar, nc.gpsimd]
            # load order tuned so S(0,0) can start after ~1.5MB: KT cols
            # [0,512) + QT cols [0,512) first, then the rest of KT, then V
            # (first PV needs it ~30us in), then remaining QT
            ne = [0]

            def load(dst, src):
                dma_engines[ne[0] % 3].dma_start(dst, src)
                ne[0] += 1

            for c in range(NDCH):
                cs = slice(c * P, (c + 1) * P)
                load(KT[c][:, 0:512], KTd[cs, 0:512])
            for c in range(NDCH):
                cs = slice(c * P, (c + 1) * P)
                load(QTh[0][c][:, 0:512], QTd[cs, 0:512])
            for blk in range(1, 4):
                for c in range(NDCH):
                    cs = slice(c * P, (c + 1) * P)
                    load(
                        KT[c][:, blk * 512 : (blk + 1) * 512],
                        KTd[cs, blk * 512 : (blk + 1) * 512],
                    )
            for c in range(NDCH):
                cs = slice(c * P, (c + 1) * P)
                load(QTh[0][c][:, 512:1024], QTd[cs, 512:1024])
            Vt = v_pool.tile([P, 16, D], F16, tag="V", name="Vt")
            for kt in range(16):
                load(Vt[:, kt, :], V[kt * P : (kt + 1) * P, :])
            for half in range(2):
                for c in range(NDCH):
                    cs = slice(c * P, (c + 1) * P)
                    load(
                        QTh[1][c][:, half * 512 : (half + 1) * 512],
                        QTd[cs, 1024 + half * 512 : 1024 + (half + 1) * 512],
                    )

            def emit_scores_softmax(t):
                qh = QTh[t // 8]
                q0 = (t % 8) * P
                nm = stat_pool.tile([P, 8], F32, tag="nm")
                sblk = stat_pool.tile([P, 8], F32, tag="sblk")
                E = e_pool.tile([P, SEG_LEN], F16, tag="E", name="E")
                bias_idx = 0
                for sb in range(4):
                    n0 = sb * 512
                    Sb = s_psum.tile([P, 512], F32, tag="S", name="Sb")
                    for d in range(NDCH):
                        nc.tensor.matmul(
                            Sb[:],
                            qh[d][:, q0 : q0 + P],
                            KT[d][:, n0 : n0 + 512],
                            start=(d == 0),
                            stop=(d == NDCH - 1),
                        )
                    # bias sub-blocks within this S-block
                    subs = [(0, 256), (256, 512)] if sb == 0 else [(0, 512)]
                    for (la, lb) in subs:
                        bi = bias_idx
                        bias_idx += 1
                        nc.vector.tensor_reduce(
                            nm[:, bi : bi + 1], Sb[:, la:lb],
                            mybir.AxisListType.X, mybir.AluOpType.max,
                            negate=True,
                        )
                        nc.scalar.activation(
                            E[:, n0 + la : n0 + lb], Sb[:, la:lb],
                            mybir.ActivationFunctionType.Exp,
                            bias=nm[:, bi : bi + 1], scale=1.0,
                            accum_out=sblk[:, bi : bi + 1],
                        )
                # combine coefficients c[:, b] for the 5 column blocks
                c = stat_pool.tile([P, 8], F32, tag="c")
                first = True
                for r in (1, 2, 4):
                    if t * P >= SEG_LEN // r:
                        continue
                    nb = RATE_BLOCKS[r]
                    nM = stat_pool.tile([P, 1], F32, tag="nM")
                    nc.vector.tensor_reduce(
                        nM[:], nm[:, :nb], mybir.AxisListType.X,
                        mybir.AluOpType.min,
                    )
                    g = stat_pool.tile([P, 8], F32, tag="g")
                    nc.scalar.activation(
                        g[:, :nb], nm[:, :nb],
                        mybir.ActivationFunctionType.Exp,
                        bias=nM[:], scale=-1.0,
                    )
                    tmp = stat_pool.tile([P, 8], F32, tag="tmp")
                    nc.vector.tensor_mul(tmp[:, :nb], g[:, :nb], sblk[:, :nb])
                    Z = stat_pool.tile([P, 1], F32, tag="Z")
                    nc.vector.tensor_reduce(
                        Z[:], tmp[:, :nb], mybir.AxisListType.X,
                        mybir.AluOpType.add,
                    )
                    iZ = stat_pool.tile([P, 1], F32, tag="iZ")
                    nc.vector.reciprocal(iZ[:], Z[:])
                    if first:
                        nc.vector.tensor_scalar_mul(c[:, :nb], g[:, :nb], iZ[:])
                        first = False
                    else:
                        nc.vector.scalar_tensor_tensor(
                            c[:, :nb], g[:, :nb], iZ[:], c[:, :nb],
                            mybir.AluOpType.mult, mybir.AluOpType.add,
                        )
                if t * P < SEG_LEN // 8:  # rate 8 = bias block 0 exactly
                    i8 = stat_pool.tile([P, 1], F32, tag="i8")
                    nc.vector.reciprocal(i8[:], sblk[:, 0:1])
                    nc.vector.tensor_add(c[:, 0:1], c[:, 0:1], i8[:])
                # W = E * c per column block (combined, pre-normalized weights)
                W = dg_pool.tile([P, SEG_LEN], F16, tag="W", name="W")
                for b, (a, e) in enumerate(BIAS_BLOCKS):
                    nc.vector.tensor_scalar_mul(
                        W[:, a:e], E[:, a:e], c[:, b : b + 1]
                    )
                return {"t": t, "W": W}

            def emit_pv(stg):
                t, W = stg["t"], stg["W"]
                Oh = [
                    o_psum.tile([P, 512], F32, tag="O", name=f"Oh{h}")
                    for h in range(2)
                ]
                # one xbar-transpose DMA produces all 16 W^T tiles:
                # WTall[j, kt, p] = W[p, kt*128 + j]
                WTall = wt_pool.tile([P, 16, P], F16, tag="wt", name="WTall")
                nc.sync.dma_start_transpose(WTall[:], W[:])
                for kt in range(16):
                    for h in range(2):
                        nc.tensor.matmul(
                            Oh[h][:],
                            WTall[:, kt, :],
                            Vt[:, kt, h * 512 : (h + 1) * 512],
                            start=(kt == 0),
                            stop=(kt == 15),
                        )

                Osb = o_pool.tile([P, D], F32, tag="Osb")
                nc.vector.tensor_copy(Osb[:, 0:512], Oh[0][:])
                nc.sync.dma_start(O[t * P : (t + 1) * P, 0:512], Osb[:, 0:512])
                nc.scalar.copy(Osb[:, 512:1024], Oh[1][:])
                nc.scalar.dma_start(
                    O[t * P : (t + 1) * P, 512:1024], Osb[:, 512:1024]
                )

            # software pipeline: PV runs two q-tiles behind scores
            pending = []
            for t in range(SEG_LEN // P):
                if len(pending) >= 2:
                    emit_pv(pending.pop(0))
                pending.append(emit_scores_softmax(t))
            for stg in pending:
                emit_pv(stg)

    _split_multi_waits(nc)
    return nc


_NC_CACHE = None

# permutation: residue classes mod 8 ordered {0},{4},{2,6},{odd} so every
# rate's strided index set is a contiguous prefix
_PERM = np.concatenate([
    np.arange(0, SEG_LEN, 8),
    np.arange(4, SEG_LEN, 8),
    np.arange(2, SEG_LEN, 4),
    np.arange(1, SEG_LEN, 2),
])
_IPERM = np.argsort(_PERM)


def make_in_maps(Q, K, V):
    Q = np.asarray(Q)
    K = np.asarray(K)
    V = np.asarray(V)
    B, S, Dm = Q.shape
    n_seg = S // SEG_LEN
    in_maps = []
    for c in range(8):
        b, g = divmod(c, n_seg)
        sl = slice(g * SEG_LEN, (g + 1) * SEG_LEN)
        in_maps.append(
            {
                "QT": np.ascontiguousarray(Q[b, sl][_PERM].T, dtype=np.float16),
                "KT": np.ascontiguousarray(K[b, sl][_PERM].T, dtype=np.float16),
                "V": np.ascontiguousarray(V[b, sl][_PERM], dtype=np.float16),
            }
        )
    return in_maps


def kernel(Q, K, V):
    global _NC_CACHE
    Q = np.asarray(Q)
    K = np.asarray(K)
    V = np.asarray(V)
    B, S, Dm = Q.shape
    n_seg = S // SEG_LEN
    assert (B, S, Dm) == (2, 8192, 1024) and n_seg == 4

    if _NC_CACHE is None:
        _NC_CACHE = build_kernel()
    nc = _NC_CACHE

    in_maps = make_in_maps(Q, K, V)
    res = run_bass_kernel_spmd(nc, in_maps, core_ids=list(range(8)))
    out = np.empty((B, S, Dm), dtype=np.float32)
    for c in range(8):
        b, g = divmod(c, n_seg)
        out[b, g * SEG_LEN : (g + 1) * SEG_LEN, :] = res.results[c]["O"][_IPERM]
    return out


if __name__ == "__main__":
    rng = np.random.default_rng(0)
    Q = rng.standard_normal((2, 8192, 1024), dtype=np.float32)
    K = rng.standard_normal((2, 8192, 1024), dtype=np.float32)
    V = rng.standard_normal((2, 8192, 1024), dtype=np.float32)
    out = kernel(Q=Q, K=K, V=V)
    print("ran ok", out.shape, out.dtype, np.abs(out).mean())


# revision 27
# speedup vs baseline: 1.0615x; 1.0615x over previous
"""Dilated attention Trainium2 kernel (combined-weights formulation).

Problem: for each (batch, segment) pair, and each dilation rate r in {1,2,4,8}:
  out_seg[::r] += softmax(Q_seg[::r] @ K_seg[::r].T) @ V_seg[::r]

Key algebra: with q/k/v rows PERMUTED so each rate's strided index set becomes
a contiguous prefix (residue classes mod 8 ordered as {0},{4},{2,6},{odd}),
every rate-r score matrix is the leading L_r x L_r block of the ONE full
2048x2048 score matrix S.  Softmax is shift-invariant per row, so a single
exp table E = exp(S - blockmax) serves all rates; each rate only needs its
own row-normalizer.  Summing the per-rate softmax(P_r)@V contributions gives
  out[p] = (E[p, :] * c_{b(j)}[p]) @ V        (one full-width PV matmul)
where c_b[p] collapses the active rates' 1/Z_r weights per column block.
PE work drops from 2.82 to 2.12 full-matmul units and the per-rate DRAM
scratch round-trip disappears.

Numerics: exp biases are per-column-block maxes with block edges on the rate
boundaries (256/512/1024), so every rate's dominant weights are O(1) in fp16.
The combine coefficients c (<= 4.0) are built in f32 from block sums and
folded into the PE transpose by replacing the identity with diag(c).

Sharding: B=2 x n_seg=4 = 8 independent (batch, segment) pairs -> one per core.
"""

import sys

if "/opt/trn_rl_repo" not in sys.path:
    sys.path.insert(0, "/opt/trn_rl_repo")

import numpy as np

import concourse.bass as bass
import concourse.mybir as mybir
from concourse import tile
from concourse.masks import make_identity
from concourse.bass_utils import run_bass_kernel_spmd

SEG_LEN = 2048
D = 1024
P = 128
NDCH = D // P  # 8 d-chunks of 128
F16 = mybir.dt.float16
F32 = mybir.dt.float32

# bias blocks: aligned to the rate boundaries 256/512/1024
BIAS_BLOCKS = [(0, 256), (256, 512), (512, 1024), (1024, 1536), (1536, 2048)]
# j-tile (128 wide) -> bias block index
GROUP_OF_JT = [0, 0, 1, 1, 2, 2, 2, 2, 3, 3, 3, 3, 4, 4, 4, 4]
# rate -> bias blocks it spans (rate 8 handled separately: single block 0 half)
RATE_BLOCKS = {1: 5, 2: 3, 4: 2}  # number of leading bias blocks

_ws_ctr = [0]


def _split_multi_waits(nc):
    """walrus in this env accepts only ONE sync-wait per instruction; move
    extras onto same-engine NoOps inserted right before the instruction."""
    for f in nc.m.functions:
        for b in f.blocks:
            out, changed = [], False
            for inst in b.instructions:
                si = inst.sync_info
                if si is not None and si.on_wait and len(si.on_wait) > 1:
                    waits = list(si.on_wait)
                    for w in waits[:-1]:
                        nop = mybir.InstNoOp(
                            name=f"waitsplit_{_ws_ctr[0]}", ins=[], outs=[]
                        )
                        _ws_ctr[0] += 1
                        nop.engine = inst.engine
                        nop.sync_info = mybir.SyncInfo(on_wait=[w], on_update=[])
                        out.append(nop)
                    si.on_wait = [waits[-1]]
                    changed = True
                out.append(inst)
            if changed:
                b.instructions = out


def build_kernel():
    # note: --enable-ldw-opt=true crashes the device (NRT_EXEC_UNIT_UNRECOVERABLE)
    # note: nc.scalar-issued xbar-transpose DMAs return wrong data in this env
    nc = bass.Bass()
    # host uploads Q,K pre-transposed ([d, l]), pre-permuted and pre-cast
    QTd = nc.dram_tensor("QT", (D, SEG_LEN), F16, kind="ExternalInput")
    KTd = nc.dram_tensor("KT", (D, SEG_LEN), F16, kind="ExternalInput")
    # V is uploaded partition-major: V[p, kt*D + d] = v_row(kt*128 + p)[d],
    # so SBUF tile loads are plain contiguous slices
    V = nc.dram_tensor("V", (P, 16 * D), F16, kind="ExternalInput")
    O = nc.dram_tensor("O", (SEG_LEN, D), F32, kind="ExternalOutput")

    with tile.TileContext(nc) as tc:
        with (
            tc.tile_pool(name="qkt", bufs=1) as qkt_pool,
            tc.tile_pool(name="vp", bufs=1) as v_pool,
            tc.tile_pool(name="ep", bufs=6) as e_pool,
            tc.tile_pool(name="wt", bufs=3) as wt_pool,
            tc.tile_pool(name="dg", bufs=6) as dg_pool,
            tc.tile_pool(name="op", bufs=3) as o_pool,
            tc.tile_pool(name="st", bufs=12) as stat_pool,
            tc.tile_pool(name="misc", bufs=1) as misc_pool,
            tc.tile_pool(name="spsum", bufs=4, space="PSUM") as s_psum,
            tc.tile_pool(name="opsum", bufs=3, space="PSUM") as o_psum,
        ):
            # ---- loads.  Per-queue DMA rings keep only ~3 transfers in
            # flight and small DMAs are latency-bound (~50GB/s/queue), so:
            # small chunks only for the critical first tiles, few BIG DMAs
            # for the bulk.  sync's queue stays clear for the transpose DMAs.
            KT = [
                qkt_pool.tile([P, SEG_LEN], F16, tag=f"KT{c}", name=f"KT{c}")
                for c in range(NDCH)
            ]
            QT = [
                qkt_pool.tile([P, SEG_LEN], F16, tag=f"QT{c}", name=f"QT{c}")
                for c in range(NDCH)
            ]
            dma_engines = [nc.sync, nc.scalar, nc.gpsimd]
            ne = [0]

            def load(dst, src):
                dma_engines[ne[0] % 3].dma_start(dst, src)
                ne[0] += 1

            # phase 1 (small, critical): keys [0,512) + queries for tiles 0-3
            for c in range(NDCH):
                cs = slice(c * P, (c + 1) * P)
                load(KT[c][:, 0:512], KTd[cs, 0:512])
            for c in range(NDCH):
                cs = slice(c * P, (c + 1) * P)
                load(QT[c][:, 0:512], QTd[cs, 0:512])
            # phase 2: first V rows (first PV chain) as one big DMA
            Vt = v_pool.tile([P, 16, D], F16, tag="V", name="Vt")
            nc.gpsimd.dma_start(Vt[:, 0:4, :], V[:, 0 : 4 * D])
            # phase 3 (big): rest of KT, then rest of QT; sync takes a couple
            # early ones then stays free for transposes
            for c in range(NDCH):
                cs = slice(c * P, (c + 1) * P)
                eng = nc.sync if c == 0 else (nc.scalar if c % 2 else nc.gpsimd)
                eng.dma_start(KT[c][:, 512:2048], KTd[cs, 512:2048])
            for c in range(NDCH):
                cs = slice(c * P, (c + 1) * P)
                eng = nc.scalar if c % 2 else nc.gpsimd
                eng.dma_start(QT[c][:, 512:2048], QTd[cs, 512:2048])
            # phase 4: rest of V
            nc.scalar.dma_start(Vt[:, 4:10, :], V[:, 4 * D : 10 * D])
            nc.gpsimd.dma_start(Vt[:, 10:16, :], V[:, 10 * D : 16 * D])

            def emit_scores_softmax(t):
                qh = QT
                q0 = t * P
                nm = stat_pool.tile([P, 8], F32, tag="nm")
                sblk = stat_pool.tile([P, 8], F32, tag="sblk")
                E = e_pool.tile([P, SEG_LEN], F16, tag="E", name="E")
                bias_idx = 0
                for sb in range(4):
                    n0 = sb * 512
                    Sb = s_psum.tile([P, 512], F32, tag="S", name="Sb")
                    for d in range(NDCH):
                        nc.tensor.matmul(
                            Sb[:],
                            qh[d][:, q0 : q0 + P],
                            KT[d][:, n0 : n0 + 512],
                            start=(d == 0),
                            stop=(d == NDCH - 1),
                        )
                    # bias sub-blocks within this S-block
                    subs = [(0, 256), (256, 512)] if sb == 0 else [(0, 512)]
                    for (la, lb) in subs:
                        bi = bias_idx
                        bias_idx += 1
                        nc.vector.tensor_reduce(
                            nm[:, bi : bi + 1], Sb[:, la:lb],
                            mybir.AxisListType.X, mybir.AluOpType.max,
                            negate=True,
                        )
                        nc.scalar.activation(
                            E[:, n0 + la : n0 + lb], Sb[:, la:lb],
                            mybir.ActivationFunctionType.Exp,
                            bias=nm[:, bi : bi + 1], scale=1.0,
                            accum_out=sblk[:, bi : bi + 1],
                        )
                # combine coefficients c[:, b] for the 5 column blocks
                c = stat_pool.tile([P, 8], F32, tag="c")
                first = True
                for r in (1, 2, 4):
                    if t * P >= SEG_LEN // r:
                        continue
                    nb = RATE_BLOCKS[r]
                    nM = stat_pool.tile([P, 1], F32, tag="nM")
                    nc.vector.tensor_reduce(
                        nM[:], nm[:, :nb], mybir.AxisListType.X,
                        mybir.AluOpType.min,
                    )
                    g = stat_pool.tile([P, 8], F32, tag="g")
                    nc.scalar.activation(
                        g[:, :nb], nm[:, :nb],
                        mybir.ActivationFunctionType.Exp,
                        bias=nM[:], scale=-1.0,
                    )
                    tmp = stat_pool.tile([P, 8], F32, tag="tmp")
                    nc.vector.tensor_mul(tmp[:, :nb], g[:, :nb], sblk[:, :nb])
                    Z = stat_pool.tile([P, 1], F32, tag="Z")
                    nc.vector.tensor_reduce(
                        Z[:], tmp[:, :nb], mybir.AxisListType.X,
                        mybir.AluOpType.add,
                    )
                    iZ = stat_pool.tile([P, 1], F32, tag="iZ")
                    nc.vector.reciprocal(iZ[:], Z[:])
                    if first:
                        nc.vector.tensor_scalar_mul(c[:, :nb], g[:, :nb], iZ[:])
                        first = False
                    else:
                        nc.vector.scalar_tensor_tensor(
                            c[:, :nb], g[:, :nb], iZ[:], c[:, :nb],
                            mybir.AluOpType.mult, mybir.AluOpType.add,
                        )
                if t * P < SEG_LEN // 8:  # rate 8 = bias block 0 exactly
                    i8 = stat_pool.tile([P, 1], F32, tag="i8")
                    nc.vector.reciprocal(i8[:], sblk[:, 0:1])
                    nc.vector.tensor_add(c[:, 0:1], c[:, 0:1], i8[:])
                # W = E * c per column block (combined, pre-normalized weights)
                W = dg_pool.tile([P, SEG_LEN], F16, tag="W", name="W")
                for b, (a, e) in enumerate(BIAS_BLOCKS):
                    nc.vector.tensor_scalar_mul(
                        W[:, a:e], E[:, a:e], c[:, b : b + 1]
                    )
                return {"t": t, "W": W}

            def emit_pv(stg):
                t, W = stg["t"], stg["W"]
                Oh = [
                    o_psum.tile([P, 512], F32, tag="O", name=f"Oh{h}")
                    for h in range(2)
                ]
                # one xbar-transpose DMA produces all 16 W^T tiles:
                # WTall[j, kt, p] = W[p, kt*128 + j]
                WTall = wt_pool.tile([P, 16, P], F16, tag="wt", name="WTall")
                nc.sync.dma_start_transpose(WTall[:], W[:])
                for kt in range(16):
                    for h in range(2):
                        nc.tensor.matmul(
                            Oh[h][:],
                            WTall[:, kt, :],
                            Vt[:, kt, h * 512 : (h + 1) * 512],
                            start=(kt == 0),
                            stop=(kt == 15),
                        )

                Osb = o_pool.tile([P, D], F32, tag="Osb")
                nc.vector.tensor_copy(Osb[:, 0:512], Oh[0][:])
                nc.scalar.dma_start(O[t * P : (t + 1) * P, 0:512], Osb[:, 0:512])
                nc.scalar.copy(Osb[:, 512:1024], Oh[1][:])
                nc.gpsimd.dma_start(
                    O[t * P : (t + 1) * P, 512:1024], Osb[:, 512:1024]
                )

            # software pipeline: PV runs four q-tiles behind scores (deep lag
            # lets the big bulk-load DMAs land before the first PV needs V)
            pending = []
            for t in range(SEG_LEN // P):
                if len(pending) >= 4:
                    emit_pv(pending.pop(0))
                pending.append(emit_scores_softmax(t))
            for stg in pending:
                emit_pv(stg)

    _split_multi_waits(nc)
    return nc


_NC_CACHE = None

# permutation: residue classes mod 8 ordered {0},{4},{2,6},{odd} so every
# rate's strided index set is a contiguous prefix
_PERM = np.concatenate([
    np.arange(0, SEG_LEN, 8),
    np.arange(4, SEG_LEN, 8),
    np.arange(2, SEG_LEN, 4),
    np.arange(1, SEG_LEN, 2),
])
_IPERM = np.argsort(_PERM)


def make_in_maps(Q, K, V):
    Q = np.asarray(Q)
    K = np.asarray(K)
    V = np.asarray(V)
    B, S, Dm = Q.shape
    n_seg = S // SEG_LEN
    in_maps = []
    for c in range(8):
        b, g = divmod(c, n_seg)
        sl = slice(g * SEG_LEN, (g + 1) * SEG_LEN)
        in_maps.append(
            {
                "QT": np.ascontiguousarray(Q[b, sl][_PERM].T, dtype=np.float16),
                "KT": np.ascontiguousarray(K[b, sl][_PERM].T, dtype=np.float16),
                "V": np.ascontiguousarray(
                    V[b, sl][_PERM]
                    .reshape(16, 128, 1024)
                    .transpose(1, 0, 2)
                    .reshape(128, 16 * 1024),
                    dtype=np.float16,
                ),
            }
        )
    return in_maps


def kernel(Q, K, V):
    global _NC_CACHE
    Q = np.asarray(Q)
    K = np.asarray(K)
    V = np.asarray(V)
    B, S, Dm = Q.shape
    n_seg = S // SEG_LEN
    assert (B, S, Dm) == (2, 8192, 1024) and n_seg == 4

    if _NC_CACHE is None:
        _NC_CACHE = build_kernel()
    nc = _NC_CACHE

    in_maps = make_in_maps(Q, K, V)
    res = run_bass_kernel_spmd(nc, in_maps, core_ids=list(range(8)))
    out = np.empty((B, S, Dm), dtype=np.float32)
    for c in range(8):
        b, g = divmod(c, n_seg)
        out[b, g * SEG_LEN : (g + 1) * SEG_LEN, :] = res.results[c]["O"][_IPERM]
    return out


if __name__ == "__main__":
    rng = np.random.default_rng(0)
    Q = rng.standard_normal((2, 8192, 1024), dtype=np.float32)
    K = rng.standard_normal((2, 8192, 1024), dtype=np.float32)
    V = rng.standard_normal((2, 8192, 1024), dtype=np.float32)
    out = kernel(Q=Q, K=K, V=V)
    print("ran ok", out.shape, out.dtype, np.abs(out).mean())
